# revision 24
# baseline (speedup 1.0000x reference)
"""Multi-head attention (B=4, N=2048, C=1024, H=16, D=64) on 8 Trainium2 cores.

Sharding: core = b*2 + hg  (b in 0..3 batches, hg in 0..1 head-groups of 8 heads).
Each core computes, for its (b, hg):
    Q^T, K^T   [512, 2048] bf16  (8 heads x 64 dims on partitions, queries free)
    V          [2048, 512] bf16  (keys on partitions) + ones column (softmax denom)
    per head pair (2 heads stacked on 128 partitions):
        S^T tiles = K_h^T.T @ Q_h^T  (bf16 matmul; keys on partitions)
        expS^T: ScalarE exp (bf16 out) or, for 2 of every 7 tiles, a DVE
                Schraudolph bit-trick exp (i16 mult+add, bitcast to bf16)
        ctx^T/den = [V_h | 1].T @ expS^T  (ones-augmented PV matmul, row 64 = denom)
        normalize: den -> DRAM -> stride-0 broadcast DMA -> fast reciprocal -> mul
    out_partial = ctx^T.T @ wo_hg^T  [2048, 1024]  (f32r matmul)
Host: out[b] = out_partial[b,hg=0] + out_partial[b,hg=1] + bo.

The QKV projection quarters and the attention blocks are emitted interleaved
in one pool region so the Tile scheduler overlaps PE-bound projection work
with ScalarE/DVE-bound softmax work. PSUM: 2 banks QKV accum + 4 banks S
double-buffer + 2 banks PV/proj accum.
S^T matmuls for the two heads of a pair are row-packed onto the 128x128 PE
array (K=64 each at base partitions 0/64) and run concurrently.
"""
import sys

sys.path.insert(0, "/opt/trn_rl_repo")

import numpy as np

import concourse.bass as bass  # noqa: F401
import concourse.tile as tile
from concourse import bacc, mybir
from concourse.bass_utils import run_bass_kernel_spmd

P = 128
B, N, C = 4, 2048, 1024
H = 16
D = 64
HG = 2                 # head groups (tensor-parallel dim)
NH = H // HG           # 8 heads per core
CH = NH * D            # 512 ctx channels per core
KO = C // P            # 8 contraction tiles for projections
NKT = N // P           # 16 key tiles
QC = 512               # query chunk (psum bank)
NQC = N // QC          # 4 query chunks
NXQ = 4                # x streamed in quarters
NQUARTER = N // NXQ
SCALE = D ** -0.5

f32 = mybir.dt.float32
f32r = mybir.dt.float32r
bf16 = mybir.dt.bfloat16
i16 = mybir.dt.int16

# Schraudolph fast-exp constants (DVE path, bf16 flavor): exp(SCALE*s) ~=
# bitcast_bf16(i16(s * (2^7/ln2 * SCALE) + (127*2^7 - 366393/2^16)))
EXP_A = float(2 ** 7 / np.log(2) * SCALE)
EXP_B = float(127 * 2 ** 7 - 366393 / 2 ** 16)

_CACHE = {}


def _build(variant="full"):
    """Build + compile the per-core Bass program (same for all 8 cores).

    variant: "full" | "nop" (overhead probe) | "qkv" (projections only) |
             "attn" (no output projection) — for ablation benchmarking.
    """
    if variant in _CACHE:
        return _CACHE[variant]

    nc = bacc.Bacc("TRN2", target_bir_lowering=False, debug=False)

    xt_d = nc.dram_tensor("xt", [KO, P, N], f32r, kind="ExternalInput").ap()
    wq_d = nc.dram_tensor("wq", [KO, P, CH], f32r, kind="ExternalInput").ap()
    wk_d = nc.dram_tensor("wk", [KO, P, CH], f32r, kind="ExternalInput").ap()
    wv_d = nc.dram_tensor("wv", [KO, P, CH], f32r, kind="ExternalInput").ap()
    wo_d = nc.dram_tensor("wo", [CH // P, P, C], f32r, kind="ExternalInput").ap()
    ones_d = nc.dram_tensor("ones", [P, 1], f32r, kind="ExternalInput").ap()
    out_d = nc.dram_tensor("out", [N, C], f32, kind="ExternalOutput").ap()

    with tile.TileContext(nc) as tc:
        with tc.tile_pool(name="persist", bufs=1) as persist:
            if variant == "nop":
                nop_t = persist.tile([P, QC], f32, tag="nop")
                nc.vector.memset(nop_t[:], 0.0)
                nc.sync.dma_start(out_d[0:P, 0:QC], nop_t[:])
            else:
                _build_body(nc, tc, persist, variant,
                            xt_d, wq_d, wk_d, wv_d, wo_d, ones_d, out_d)

    nc.compile()
    _CACHE[variant] = nc
    return nc


def _build_body(nc, tc, persist, variant, xt_d, wq_d, wk_d, wv_d, wo_d, ones_d, out_d):
    qt = persist.tile([P, CH // P, N], bf16, tag="qt")
    kt = persist.tile([P, CH // P, N], bf16, tag="kt")
    v = persist.tile([P, NKT, NH, D + 1], bf16, tag="v")
    ones = persist.tile([P, 1], f32r, tag="ones")
    nc.sync.dma_start(ones[:], ones_d[:])

    with (
        tc.tile_pool(name="px", bufs=2) as px,
        tc.tile_pool(name="pw", bufs=1) as pw,
        tc.tile_pool(name="pwo", bufs=1) as pwo,
        tc.tile_pool(name="pe", bufs=8) as pe_pool,
        tc.tile_pool(name="pden", bufs=2) as pden,
        tc.tile_pool(name="pout", bufs=4) as pout,
        tc.tile_pool(name="pctx", bufs=2) as pctx,
        tc.tile_pool(name="pdram", bufs=2, space="DRAM") as pdram,
        tc.tile_pool(name="pst", bufs=2, space="PSUM") as pst,
    ):
        wts = {}

        def load_w(name, wd):
            w = pw.tile([P, KO, CH], f32r, tag=f"w{name}", name=f"w_{name}")
            for ko in range(KO):
                nc.sync.dma_start(w[:, ko, :], wd[ko])
            wts[name] = w

        state = {"ecount": 0}

        def emit_exp(e_tile, st_tile):
            """2 of every 7 exp instructions run on DVE (Schraudolph bit
            trick); the rest on ACT. e_tile is i16, read back as bf16."""
            c = state["ecount"]
            if c % 7 in (2, 5):
                nc.vector.tensor_scalar(
                    out=e_tile[:], in0=st_tile[:], scalar1=EXP_A,
                    scalar2=EXP_B, op0=mybir.AluOpType.mult,
                    op1=mybir.AluOpType.add)
            else:
                nc.scalar.activation(e_tile[:].bitcast(bf16), st_tile[:],
                                     mybir.ActivationFunctionType.Exp,
                                     scale=SCALE)
            state["ecount"] = c + 1

        def load_x(quarter):
            hsl = slice(quarter * NQUARTER, (quarter + 1) * NQUARTER)
            xt = px.tile([P, KO, NQUARTER], f32r, tag="x", name=f"x_{quarter}")
            for ko in range(KO):
                nc.sync.dma_start(xt[:, ko, :], xt_d[ko, :, hsl])
            return xt

        def emit_kv_quarter(quarter, psum1):
            """V + K projections for one 512-query quarter of x."""
            hsl = slice(quarter * NQUARTER, (quarter + 1) * NQUARTER)
            xt = load_x(quarter)
            # V first (attention needs V tiles before any PV work)
            for i in range(NQUARTER // P):
                ikt = quarter * (NQUARTER // P) + i
                ps = psum1.tile([P, CH], f32, tag="ps1")
                for ko in range(KO):
                    nc.tensor.matmul(
                        ps[:], xt[:, ko, i * P:(i + 1) * P], wts["v"][:, ko, :],
                        start=(ko == 0), stop=(ko == KO - 1))
                nc.vector.tensor_copy(
                    v[:, ikt, :, 0:D], ps[:].rearrange("p (h d) -> p h d", d=D))
            # ones column (col 64) for this quarter's key tiles
            i0 = quarter * (NQUARTER // P)
            nc.vector.tensor_copy(
                v[:, i0:i0 + NQUARTER // P, :, D:D + 1],
                ones[:].unsqueeze(1).unsqueeze(1)
                       .broadcast_to([P, NQUARTER // P, NH, 1]))
            for mt in range(CH // P):
                ps = psum1.tile([P, CH], f32, tag="ps1")
                for ko in range(KO):
                    nc.tensor.matmul(
                        ps[:], wts["k"][:, ko, mt * P:(mt + 1) * P], xt[:, ko, :],
                        start=(ko == 0), stop=(ko == KO - 1))
                nc.vector.tensor_copy(kt[:, mt, hsl], ps[:])

        def emit_q_quarter(quarter, ppv):
            """Q projection for one quarter (x reloaded; runs interleaved
            with attention of earlier query chunks; accumulators share the
            attention PV bank pool)."""
            hsl = slice(quarter * NQUARTER, (quarter + 1) * NQUARTER)
            xt = load_x(quarter)
            for mt in range(CH // P):
                ps = ppv.tile([P, CH], f32, tag="pvA" if mt % 2 == 0 else "pvB",
                              name=f"q_ps_{quarter}_{mt}")
                for ko in range(KO):
                    nc.tensor.matmul(
                        ps[:], wts["q"][:, ko, mt * P:(mt + 1) * P], xt[:, ko, :],
                        start=(ko == 0), stop=(ko == KO - 1))
                nc.vector.tensor_copy(qt[:, mt, hsl], ps[:])

        def emit_attn(qc, ctx, ppv, gap=None):
            """gap: optional list of thunks; gap[i] is emitted after head
            pair i (fills PE slack with projection work of an earlier qc)."""
            qsl = slice(qc * QC, (qc + 1) * QC)
            for hp in range(CH // P):    # head pair = partition tile of qt/kt
                psA = ppv.tile([P, QC], f32, tag="pvA")
                psB = ppv.tile([P, QC], f32, tag="pvB")
                for ik in range(NKT):
                    ksl = slice(ik * P, (ik + 1) * P)
                    stA = pst.tile([P, QC], f32, tag="stA")
                    stB = pst.tile([P, QC], f32, tag="stB")
                    nc.tensor.matmul(stA[:], kt[0:D, hp, ksl],
                                     qt[0:D, hp, qsl], start=True, stop=True)
                    nc.tensor.matmul(stB[:], kt[D:P, hp, ksl],
                                     qt[D:P, hp, qsl], start=True, stop=True)
                    eA = pe_pool.tile([P, QC], i16, tag="eA")
                    eB = pe_pool.tile([P, QC], i16, tag="eB")
                    emit_exp(eA, stA)
                    emit_exp(eB, stB)
                    first, last = ik == 0, ik == NKT - 1
                    nc.tensor.matmul(psA[0:D + 1, :], v[:, ik, 2 * hp, :],
                                     eA[:].bitcast(bf16), start=first, stop=last)
                    nc.tensor.matmul(psB[0:D + 1, :], v[:, ik, 2 * hp + 1, :],
                                     eB[:].bitcast(bf16), start=first, stop=last)
                # softmax denominators: stage to DRAM (same-partition copy
                # first), broadcast back across partitions via stride-0 DMA,
                # batched fast reciprocal, then normalize into the ctx tile.
                den = pden.tile([P, 2, QC], f32, tag="den")
                nc.vector.tensor_copy(den[D:D + 1, 0, :], psA[D:D + 1, :])
                nc.vector.tensor_copy(den[D:D + 1, 1, :], psB[D:D + 1, :])
                den_dr = pdram.tile([2, QC], f32, tag="den_dr")
                nc.sync.dma_start(den_dr[:], den[D:D + 1, :, :])
                bcr = pden.tile([P, QC], f32, tag="bcr")
                nc.sync.dma_start(bcr[0:D, :], den_dr[0].partition_broadcast(D))
                nc.sync.dma_start(bcr[D:P, :], den_dr[1].partition_broadcast(D))
                bc = pden.tile([P, QC], f32, tag="bc")
                nc.vector.reciprocal_approx_fast(bc[:], bcr[:])
                nc.vector.tensor_mul(ctx[0:D, hp, :], psA[0:D, :], bc[0:D, :])
                nc.vector.tensor_mul(ctx[D:P, hp, :], psB[0:D, :], bc[D:P, :])
                if gap is not None and hp < len(gap):
                    gap[hp]()

        def emit_proj_part(qc, ctx, ppv, qi):
            """Output projection for one 128-query tile of chunk qc."""
            for nt in range(C // QC):
                tag = "pvA" if (2 * qi + nt) % 2 == 0 else "pvB"
                ps = ppv.tile([P, QC], f32, tag=tag,
                              name=f"proj_{qc}_{qi}_{nt}")
                for ct in range(CH // P):
                    nc.tensor.matmul(
                        ps[:], ctx[:, ct, qi * P:(qi + 1) * P],
                        wo[:, ct, nt * QC:(nt + 1) * QC],
                        start=(ct == 0), stop=(ct == CH // P - 1))
                ot = pout.tile([P, QC], f32, tag="ot")
                nc.vector.tensor_copy(ot[:], ps[:])
                qt_i = 4 * qc + qi
                nc.sync.dma_start(
                    out_d[qt_i * P:(qt_i + 1) * P, nt * QC:(nt + 1) * QC],
                    ot[:])

        def emit_proj(qc, ctx, ppv):
            for qi in range(4):          # 128-query tiles within this chunk
                emit_proj_part(qc, ctx, ppv, qi)

        def proj_gaps(qc, ctx, ppv):
            return [lambda qi=qi: emit_proj_part(qc, ctx, ppv, qi)
                    for qi in range(4)]

        load_w("v", wv_d)
        load_w("k", wk_d)
        with tc.tile_pool(name="psum1", bufs=2, space="PSUM") as psum1:
            for quarter in range(NXQ):
                emit_kv_quarter(quarter, psum1)
                if quarter == 0:
                    load_w("q", wq_d)
                    wo = pwo.tile([P, CH // P, C], f32r, tag="wo", name="wo_sb")
                    for ct in range(CH // P):
                        nc.sync.dma_start(wo[:, ct, :], wo_d[ct])

        with tc.tile_pool(name="ppv", bufs=2, space="PSUM") as ppv:
            emit_q_quarter(0, ppv)
            if variant == "qkv":
                for quarter in range(1, NXQ):
                    emit_q_quarter(quarter, ppv)
                ot = persist.tile([P, QC], f32, tag="dump")
                nc.vector.tensor_copy(
                    ot[:].rearrange("p (h d) -> p h d", d=D), v[:, 0, :, 0:D])
                nc.sync.dma_start(out_d[0:P, 0:QC], ot[:])
                return

            ctxs = {}
            ctxs[0] = pctx.tile([P, CH // P, QC], f32r, tag="ctx", name="ctx0")
            emit_attn(0, ctxs[0], ppv)
            emit_q_quarter(1, ppv)
            ctxs[1] = pctx.tile([P, CH // P, QC], f32r, tag="ctx", name="ctx1")
            emit_attn(1, ctxs[1], ppv)
            emit_q_quarter(2, ppv)
            emit_q_quarter(3, ppv)
            if variant == "attn":
                ctxs[2] = pctx.tile([P, CH // P, QC], f32r, tag="ctx", name="ctx2")
                emit_attn(2, ctxs[2], ppv)
                ctxs[3] = pctx.tile([P, CH // P, QC], f32r, tag="ctx", name="ctx3")
                emit_attn(3, ctxs[3], ppv)
                ot = persist.tile([P, QC], f32, tag="dump")
                nc.vector.tensor_copy(ot[:], ctxs[3][:, 0, :].bitcast(f32))
                nc.sync.dma_start(out_d[0:P, 0:QC], ot[:])
                return
            emit_proj(0, ctxs[0], ppv)
            ctxs[2] = pctx.tile([P, CH // P, QC], f32r, tag="ctx", name="ctx2")
            emit_attn(2, ctxs[2], ppv)
            emit_proj(1, ctxs[1], ppv)
            ctxs[3] = pctx.tile([P, CH // P, QC], f32r, tag="ctx", name="ctx3")
            emit_attn(3, ctxs[3], ppv)
            emit_proj(2, ctxs[2], ppv)
            emit_proj(3, ctxs[3], ppv)


def _prepare_in_maps(x, wq, wk, wv, wo):
    x = np.ascontiguousarray(np.asarray(x, dtype=np.float32))
    ws = {}
    for hg in range(HG):
        sl = slice(hg * CH, (hg + 1) * CH)
        ws[hg] = {
            "wq": np.ascontiguousarray(np.asarray(wq)[sl, :].T).reshape(KO, P, CH),
            "wk": np.ascontiguousarray(np.asarray(wk)[sl, :].T).reshape(KO, P, CH),
            "wv": np.ascontiguousarray(np.asarray(wv)[sl, :].T).reshape(KO, P, CH),
            "wo": np.ascontiguousarray(np.asarray(wo)[:, sl].T).reshape(CH // P, P, C),
        }
    ones = np.ones((P, 1), dtype=np.float32)
    in_maps = []
    for core in range(8):
        b, hg = core // HG, core % HG
        xt = np.ascontiguousarray(x[b].T).reshape(KO, P, N)
        m = {"xt": xt, "ones": ones}
        m.update(ws[hg])
        in_maps.append(m)
    return in_maps


def kernel(x, wq, wk, wv, wo, bo):
    nc = _build()
    in_maps = _prepare_in_maps(x, wq, wk, wv, wo)
    res = run_bass_kernel_spmd(nc, in_maps, core_ids=list(range(8)))
    bo = np.asarray(bo, dtype=np.float32)
    out = np.empty((B, N, C), dtype=np.float32)
    for b in range(B):
        out[b] = res.results[2 * b]["out"] + res.results[2 * b + 1]["out"] + bo
    return out
